# revision 1
# baseline (speedup 1.0000x reference)
"""GQA kernel for Trainium2, 8 NeuronCores.

Sharding: 2 batches x 4 head-shards. Each core handles one batch and
2 KV groups (= 8 Q heads, 512 of the 2048 head-concat columns).
Per core the out-projection produces a partial [S, D] sum; the host
adds the 4 partials per batch (the "all-reduce after out_proj") + bo.

Device-side math per core (b = batch, columns c0 = shard*512):
  qT2[pr] = (x_b @ Wq[:, c0+128pr : +128] + bq).T      [128, S]  (head pair)
  kT2[g]  = ((x_b @ Wk[:, ...] + bk) / 8).T, duplicated on both
            partition halves so either q-head parity can use it  [128, S]
  v       = x_b @ Wv + bv, stored per key-chunk as [64 v_g | 1]  [128, 16*130]
  scT     = kT chunk^T x qT  (keys on partitions)               [128, 512]
  eT      = exp(scT)   (no max subtraction: scores ~ N(0,1))
  ctxT    = [v_g | 1]^T @ eT -> rows 0..63 ctx^T, row 64 = softmax sums
  ctxT'   = ctxT * (1/sums)  (broadcast via K=1 PE matmul)
  y_part  = sum_pr ctxT2'[pr]^T @ Wo[...]                        [S, D]

All matmul inputs are float32r (FP22 single-pass, 1 cycle/row).
x is pre-transposed on the host so no on-device x transpose is needed.
"""

import sys

sys.path.insert(0, "/opt/trn_rl_repo")

import numpy as np

N_CORES = 8
S = 2048  # sequence length
D = 2048  # d_model
HD = 64  # head dim
HL = 8  # local Q heads per core
GL = 2  # local KV groups per core
CPS = 512  # q/out columns per shard
KPS = 128  # kv columns per shard
SCALE = 1.0 / 8.0  # 1/sqrt(HD)

_CACHE = {}


def _build_bass():
    import concourse.bass as bass
    import concourse.bacc as bacc
    import concourse.mybir as mybir
    import concourse.tile as tile
    from concourse.masks import make_identity

    f32 = mybir.dt.float32
    f32r = mybir.dt.float32r
    bf16 = mybir.dt.bfloat16
    ALU = mybir.AluOpType
    ACTF = mybir.ActivationFunctionType

    nc = bacc.Bacc("TRN2", target_bir_lowering=False)

    xT = nc.dram_tensor("xT", [D, S], f32r, kind="ExternalInput")
    Wq = nc.dram_tensor("Wq", [D, CPS], f32r, kind="ExternalInput")
    Wk = nc.dram_tensor("Wk", [D, KPS], f32r, kind="ExternalInput")
    Wv = nc.dram_tensor("Wv", [D, KPS], f32r, kind="ExternalInput")
    Wo = nc.dram_tensor("Wo", [CPS, D], f32r, kind="ExternalInput")
    bq = nc.dram_tensor("bq", [CPS], f32, kind="ExternalInput")
    bk = nc.dram_tensor("bk", [KPS], f32, kind="ExternalInput")
    bv = nc.dram_tensor("bv", [KPS], f32, kind="ExternalInput")
    y = nc.dram_tensor("y", [S, D], f32, kind="ExternalOutput")

    DC = D // 128  # 16 contraction chunks for projections
    SC = S // 128  # 16 key chunks
    QT = S // 128  # 16 query row-tiles
    QB = 4  # query blocks of 512 in attention
    QBS = S // QB

    with tile.TileContext(nc) as tc:
        with tc.tile_pool(name="persist", bufs=1) as pp:
            # ---- persistent SBUF tensors (per-partition KB in comments) ----
            qT2 = [pp.tile([128, S], f32r, name=f"qT{p}", tag=f"qT{p}") for p in range(4)]  # 32
            kT2 = [pp.tile([128, S], f32r, name=f"kT{g}", tag=f"kT{g}") for g in range(GL)]  # 16
            # v with a ones column appended per group: 16 chunks x ([64 v|1] x2)
            v_sb = pp.tile([128, SC * 130], f32r, tag="v_sb")  # 8.1
            ctxT2 = [pp.tile([128, S], f32r, name=f"ctxT{p}", tag=f"ctxT{p}") for p in range(4)]  # 32
            bqs = [pp.tile([128, 1], f32, name=f"bq{t}", tag=f"bq{t}") for t in range(4)]
            bks = pp.tile([128, 1], f32, tag="bks")
            bvs = pp.tile([128, 1], f32, tag="bvs")
            ident = pp.tile([128, 128], f32r, tag="ident")
            vones = pp.tile([128, 1], f32, tag="vones")
            ident_f32 = pp.tile([128, 128], f32, tag="ident_f32")

            nc.gpsimd.memset(vones[:], 1.0)
            for k in range(2 * SC):
                nc.vector.tensor_copy(v_sb[:, 64 + 65 * k : 65 + 65 * k], vones[:])
            make_identity(nc, ident_f32[:])
            nc.vector.tensor_copy(ident[:], ident_f32[:])

            for t in range(4):
                nc.sync.dma_start(bqs[t][:], bq[t * 128 : (t + 1) * 128])
            nc.sync.dma_start(bks[:], bk[:])
            nc.sync.dma_start(bvs[:], bv[:])
            # pre-scale bk by 1/8 (k is scaled so scores = q.k/8)
            nc.vector.tensor_scalar_mul(bks[:], bks[:], SCALE)

            # ---- phase A: projections ----
            # All of Wq|Wk|Wv resident: 12 KB/part. One xT pass feeds
            # 6 parallel PSUM accumulators (qT pairs x4, kT, vT).
            with (
                tc.tile_pool(name="wall", bufs=1) as wp,
                tc.tile_pool(name="stA", bufs=12) as st,
                tc.tile_pool(name="psA", bufs=1, space=bass.MemorySpace.PSUM) as psA,
                tc.tile_pool(name="psT", bufs=2, space=bass.MemorySpace.PSUM) as psT,
            ):
                Wall = [wp.tile([128, 768], f32r, name=f"wall{i}", tag=f"wall{i}") for i in range(DC)]
                for dc in range(DC):
                    rs = slice(dc * 128, (dc + 1) * 128)
                    nc.sync.dma_start(Wall[dc][:, 0:512], Wq[rs, :])
                    nc.sync.dma_start(Wall[dc][:, 512:640], Wk[rs, :])
                    nc.sync.dma_start(Wall[dc][:, 640:768], Wv[rs, :])

                for sq in range(4):
                    s0 = sq * 512
                    pss = [
                        psA.tile([128, 512], f32, name=f"proj{ct}_{sq}", tag=f"proj{ct}")
                        for ct in range(6)
                    ]
                    for dc in range(DC):
                        xt = st.tile([128, 512], f32r, tag="xt")
                        nc.sync.dma_start(xt[:], xT[dc * 128 : (dc + 1) * 128, s0 : s0 + 512])
                        for ct in range(6):
                            nc.tensor.matmul(
                                pss[ct][:],
                                Wall[dc][:, ct * 128 : (ct + 1) * 128],
                                xt[:],
                                start=(dc == 0),
                                stop=(dc == DC - 1),
                            )
                    sl = slice(s0, s0 + 512)
                    for ct in range(4):
                        nc.vector.tensor_scalar_add(qT2[ct][:, sl], pss[ct][:], bqs[ct][:])
                    for g in range(GL):
                        gs = slice(g * 64, (g + 1) * 64)
                        for half in range(2):
                            hs = slice(half * 64, (half + 1) * 64)
                            nc.vector.tensor_scalar(
                                kT2[g][hs, sl],
                                pss[4][gs, :],
                                SCALE,
                                bks[gs, :],
                                op0=ALU.mult,
                                op1=ALU.add,
                            )
                    vt = st.tile([128, 512], f32r, tag="vt")
                    nc.vector.tensor_scalar_add(vt[:], pss[5][:], bvs[:])
                    for c4 in range(4):
                        tck = sq * 4 + c4
                        tp = psT.tile([128, 128], f32r, tag="vtp")
                        nc.tensor.transpose(tp[:], vt[:, c4 * 128 : (c4 + 1) * 128], ident[:])
                        for g in range(GL):
                            nc.vector.tensor_copy(
                                v_sb[:, tck * 130 + g * 65 : tck * 130 + g * 65 + 64],
                                tp[:, g * 64 : (g + 1) * 64],
                            )

            # ---- phase B: attention ----
            with (
                tc.tile_pool(name="psS", bufs=3, space=bass.MemorySpace.PSUM) as psS,
                tc.tile_pool(name="psC", bufs=2, space=bass.MemorySpace.PSUM) as psC,
                tc.tile_pool(name="eT", bufs=2) as ep,
                tc.tile_pool(name="rc", bufs=2) as rp,
            ):
                for h in range(HL):
                    g = h // 4
                    pr = h // 2
                    po = (h % 2) * 64
                    ph = slice(po, po + 64)
                    for qb in range(QB):
                        qsl = slice(qb * QBS, (qb + 1) * QBS)
                        eT = ep.tile([128, SC * QBS], f32r, tag="eT")
                        ctx = psC.tile([65, QBS], f32, tag="ctx")
                        for kc2 in range(SC // 2):
                            sc_ps = psS.tile([128, 1024], f32, tag="sc")
                            for half in range(2):
                                kc = kc2 * 2 + half
                                nc.tensor.matmul(
                                    sc_ps[:, half * QBS : (half + 1) * QBS],
                                    kT2[g][ph, kc * 128 : (kc + 1) * 128],
                                    qT2[pr][ph, qsl],
                                    start=True,
                                    stop=True,
                                )
                            nc.scalar.activation(
                                eT[:, kc2 * 1024 : (kc2 + 1) * 1024],
                                sc_ps[:],
                                ACTF.Exp,
                            )
                            for half in range(2):
                                kc = kc2 * 2 + half
                                nc.tensor.matmul(
                                    ctx[:],
                                    v_sb[:, kc * 130 + g * 65 : kc * 130 + (g + 1) * 65],
                                    eT[:, kc * QBS : (kc + 1) * QBS],
                                    start=(kc == 0),
                                    stop=(kc == SC - 1),
                                )
                        recip = rp.tile([1, QBS], f32r, tag="recip")
                        with nc.allow_low_precision(reason="f32r is 4-byte"):
                            nc.vector.reciprocal(recip[:], ctx[64:65, :])
                        bc = rp.tile([64, QBS], f32r, tag="bc")
                        nc.gpsimd.partition_broadcast(bc[:], recip[:])
                        nc.vector.tensor_tensor(
                            out=ctxT2[pr][ph, qsl],
                            in0=ctx[0:64, :],
                            in1=bc[:],
                            op=ALU.mult,
                        )

            # ---- phase C: out projection (partial sum over local heads) ----
            with (
                tc.tile_pool(name="psO", bufs=2, space=bass.MemorySpace.PSUM) as psO,
                tc.tile_pool(name="stC", bufs=3) as st,
                tc.tile_pool(name="woP", bufs=1) as wop,
            ):
                Wo_sb = [wop.tile([128, D], f32r, name=f"wo{p}", tag=f"wo{p}") for p in range(4)]
                for p in range(4):
                    nc.sync.dma_start(Wo_sb[p][:], Wo[p * 128 : (p + 1) * 128, :])
                for qt in range(QT):
                    ops = psO.tile([128, D], f32, tag="out")
                    for p in range(4):
                        for nn in range(4):
                            nc.tensor.matmul(
                                ops[:, nn * 512 : (nn + 1) * 512],
                                ctxT2[p][:, qt * 128 : (qt + 1) * 128],
                                Wo_sb[p][:, nn * 512 : (nn + 1) * 512],
                                start=(p == 0),
                                stop=(p == 3),
                            )
                    osb = st.tile([128, D], f32, tag="osb")
                    nc.any.tensor_copy(osb[:], ops[:])
                    nc.sync.dma_start(y[qt * 128 : (qt + 1) * 128, :], osb[:])

    nc.compile()
    return nc


def _get_nc():
    if "nc" not in _CACHE:
        _CACHE["nc"] = _build_bass()
    return _CACHE["nc"]


def make_in_maps(x, Wq, bq, Wk, bk, Wv, bv, Wo):
    xTb = [np.ascontiguousarray(x[b].T) for b in range(2)]
    in_maps = []
    for c in range(N_CORES):
        b, sh = divmod(c, 4)
        in_maps.append(
            {
                "xT": xTb[b],
                "Wq": np.ascontiguousarray(Wq[:, sh * CPS : (sh + 1) * CPS]),
                "Wk": np.ascontiguousarray(Wk[:, sh * KPS : (sh + 1) * KPS]),
                "Wv": np.ascontiguousarray(Wv[:, sh * KPS : (sh + 1) * KPS]),
                "Wo": np.ascontiguousarray(Wo[sh * CPS : (sh + 1) * CPS, :]),
                "bq": np.ascontiguousarray(bq[sh * CPS : (sh + 1) * CPS]),
                "bk": np.ascontiguousarray(bk[sh * KPS : (sh + 1) * KPS]),
                "bv": np.ascontiguousarray(bv[sh * KPS : (sh + 1) * KPS]),
            }
        )
    return in_maps


def kernel(x, Wq, bq, Wk, bk, Wv, bv, Wo, bo):
    from concourse.bass_utils import run_bass_kernel_spmd

    x = np.asarray(x, dtype=np.float32)
    Wq = np.asarray(Wq, dtype=np.float32)
    Wk = np.asarray(Wk, dtype=np.float32)
    Wv = np.asarray(Wv, dtype=np.float32)
    Wo = np.asarray(Wo, dtype=np.float32)
    bq = np.asarray(bq, dtype=np.float32)
    bk = np.asarray(bk, dtype=np.float32)
    bv = np.asarray(bv, dtype=np.float32)
    bo = np.asarray(bo, dtype=np.float32)

    in_maps = make_in_maps(x, Wq, bq, Wk, bk, Wv, bv, Wo)
    nc = _get_nc()
    res = run_bass_kernel_spmd(nc, in_maps, core_ids=list(range(N_CORES)))
    out = np.zeros((2, S, D), dtype=np.float32)
    for c in range(N_CORES):
        b = c // 4
        out[b] += res.results[c]["y"]
    out += bo
    return out



# revision 8
# speedup vs baseline: 1.1453x; 1.1453x over previous
"""GQA kernel for Trainium2, 8 NeuronCores.

Sharding: 2 batches x 4 head-shards. Each core handles one batch and
2 KV groups (= 8 Q heads, 512 of the 2048 head-concat columns).
Per core the out-projection produces a partial [S, D] sum; the host
adds the 4 partials per batch (the "all-reduce after out_proj") + bo.

All matmul operands are fp16 (full-rate PE at 2.4 GHz; f32r runs the
array at half clock). Score matmuls contract over head_dim=64, so two
heads of a pair run CONCURRENTLY as row-tiles (rows 0-63 / 64-127) to
keep the whole PE array busy. Scores land in fp16 PSUM [128, 2048]
tiles (2 banks) so exp() runs as few, wide ACT instructions.

Device-side math per core (b = batch, columns c0 = shard*512):
  qT2[pr] = (x_b @ Wq[:, c0+128pr : +128] + bq).T        [128, S]  (head pair)
  kT2[g]  = ((x_b @ Wk[:, ...] + bk)).T, duplicated on both
            partition halves so either q-head parity can use it  [128, S]
  v       = x_b @ Wv + bv, stored per key-chunk as [64 v_g | 1]  [128, 16*130]
  scT     = kT chunk^T x qT  (keys on partitions)               [128, 512]
  eT      = exp(scT / 8)   (no max subtraction: scores ~ N(0,1))
  ctxT    = [v_g | 1]^T @ eT -> rows 0..63 ctx^T, row 64 = softmax sums
  ctxT'   = ctxT * (1/sums)  (gpsimd partition broadcast + DVE mult)
  y_part  = sum_pr ctxT2'[pr]^T @ Wo[...]                        [S, D]

x is pre-transposed + fp16-cast on the host so no on-device x
transpose is needed.
"""

import sys

sys.path.insert(0, "/opt/trn_rl_repo")

import numpy as np

N_CORES = 8
S = 2048  # sequence length
D = 2048  # d_model
HD = 64  # head dim
HL = 8  # local Q heads per core
GL = 2  # local KV groups per core
CPS = 512  # q/out columns per shard
KPS = 128  # kv columns per shard
SCALE = 1.0 / 8.0  # 1/sqrt(HD)

_CACHE = {}


def _build_bass():
    import concourse.bass as bass
    import concourse.bacc as bacc
    import concourse.mybir as mybir
    import concourse.tile as tile
    from concourse.masks import make_identity

    f32 = mybir.dt.float32
    f32r = mybir.dt.float32r
    f16 = mybir.dt.float16
    ALU = mybir.AluOpType
    ACTF = mybir.ActivationFunctionType

    nc = bacc.Bacc("TRN2", target_bir_lowering=False)

    xT = nc.dram_tensor("xT", [D, S], f16, kind="ExternalInput")
    Wq = nc.dram_tensor("Wq", [D, CPS], f16, kind="ExternalInput")
    Wk = nc.dram_tensor("Wk", [D, KPS], f16, kind="ExternalInput")
    Wv = nc.dram_tensor("Wv", [D, KPS], f16, kind="ExternalInput")
    Wo = nc.dram_tensor("Wo", [CPS, D], f16, kind="ExternalInput")
    bq = nc.dram_tensor("bq", [CPS], f32, kind="ExternalInput")
    bk = nc.dram_tensor("bk", [KPS], f32, kind="ExternalInput")
    bv = nc.dram_tensor("bv", [KPS], f32, kind="ExternalInput")
    y = nc.dram_tensor("y", [S, D], f32, kind="ExternalOutput")

    DC = D // 128  # 16 contraction chunks for projections
    SC = S // 128  # 16 key chunks
    QT = S // 128  # 16 query row-tiles
    QB = 4  # query blocks of 512 in attention
    QBS = S // QB
    KGRP = 2  # key chunks per exp group (psum tile = 2 banks f32)
    NGRP = SC // KGRP  # exp groups per (head, qb)

    with tile.TileContext(nc) as tc:
        with tc.tile_pool(name="persist", bufs=1) as pp:
            # ---- persistent SBUF tensors (per-partition KB in comments) ----
            qT2 = [pp.tile([128, S], f16, name=f"qT{p}", tag=f"qT{p}") for p in range(4)]  # 16
            kT2 = [pp.tile([128, S], f16, name=f"kT{g}", tag=f"kT{g}") for g in range(GL)]  # 8
            # v with a ones column appended per group: 16 chunks x ([64 v|1] x2)
            v_sb = pp.tile([128, SC * 130], f16, tag="v_sb")  # 4.1
            ctxT2 = [pp.tile([128, S], f16, name=f"ctxT{p}", tag=f"ctxT{p}") for p in range(4)]  # 16
            bqs = [pp.tile([128, 1], f32, name=f"bq{t}", tag=f"bq{t}") for t in range(4)]
            bks = pp.tile([128, 1], f32, tag="bks")
            bvs = pp.tile([128, 1], f32, tag="bvs")
            ident = pp.tile([128, 128], f16, tag="ident")
            vones = pp.tile([128, 1], f16, tag="vones")
            ident_f32 = pp.tile([128, 128], f32, tag="ident_f32")

            nc.gpsimd.memset(vones[:], 1.0)
            for k in range(2 * SC):
                nc.vector.tensor_copy(v_sb[:, 64 + 65 * k : 65 + 65 * k], vones[:])
            make_identity(nc, ident_f32[:])
            nc.vector.tensor_copy(ident[:], ident_f32[:])

            for t in range(4):
                nc.sync.dma_start(bqs[t][:], bq[t * 128 : (t + 1) * 128])
            nc.sync.dma_start(bks[:], bk[:])
            nc.sync.dma_start(bvs[:], bv[:])

            # ---- phase A: projections ----
            # All of Wq|Wk|Wv resident: 24 KB/part fp16. One xT pass feeds
            # 6 parallel PSUM accumulators (qT pairs x4, kT, vT).
            with (
                tc.tile_pool(name="wall", bufs=1) as wp,
                tc.tile_pool(name="stA", bufs=8) as st,
                tc.tile_pool(name="psA", bufs=1, space=bass.MemorySpace.PSUM) as psA,
                tc.tile_pool(name="psT", bufs=2, space=bass.MemorySpace.PSUM) as psT,
            ):
                Wall = [wp.tile([128, 768], f16, name=f"wall{i}", tag=f"wall{i}") for i in range(DC)]
                for dc in range(DC):
                    rs = slice(dc * 128, (dc + 1) * 128)
                    nc.sync.dma_start(Wall[dc][:, 0:512], Wq[rs, :])
                    nc.sync.dma_start(Wall[dc][:, 512:640], Wk[rs, :])
                    nc.sync.dma_start(Wall[dc][:, 640:768], Wv[rs, :])

                for sq in range(4):
                    s0 = sq * 512
                    pss = [
                        psA.tile([128, 512], f32, name=f"proj{ct}_{sq}", tag=f"proj{ct}")
                        for ct in range(6)
                    ]
                    for dc in range(DC):
                        xt = st.tile([128, 512], f16, tag="xt")
                        nc.sync.dma_start(xt[:], xT[dc * 128 : (dc + 1) * 128, s0 : s0 + 512])
                        for ct in range(6):
                            nc.tensor.matmul(
                                pss[ct][:],
                                Wall[dc][:, ct * 128 : (ct + 1) * 128],
                                xt[:],
                                start=(dc == 0),
                                stop=(dc == DC - 1),
                            )
                    sl = slice(s0, s0 + 512)
                    for ct in range(4):
                        nc.vector.tensor_scalar_add(qT2[ct][:, sl], pss[ct][:], bqs[ct][:])
                    for g in range(GL):
                        gs = slice(g * 64, (g + 1) * 64)
                        for half in range(2):
                            hs = slice(half * 64, (half + 1) * 64)
                            nc.vector.tensor_scalar_add(
                                kT2[g][hs, sl], pss[4][gs, :], bks[gs, :]
                            )
                    vt = st.tile([128, 512], f16, tag="vt")
                    nc.vector.tensor_scalar_add(vt[:], pss[5][:], bvs[:])
                    for c4 in range(4):
                        tck = sq * 4 + c4
                        tp = psT.tile([128, 128], f16, tag="vtp")
                        nc.tensor.transpose(tp[:], vt[:, c4 * 128 : (c4 + 1) * 128], ident[:])
                        for g in range(GL):
                            nc.vector.tensor_copy(
                                v_sb[:, tck * 130 + g * 65 : tck * 130 + g * 65 + 64],
                                tp[:, g * 64 : (g + 1) * 64],
                            )

            # ---- phase B: attention ----
            # Loop: qb outer, head-pair (pr) inner. The two heads of a
            # pair run their score matmuls concurrently as row-tiles
            # (rows 0:64 and 64:128), keeping the full PE array active.
            with (
                tc.tile_pool(name="psS", bufs=3, space=bass.MemorySpace.PSUM) as psS,
                tc.tile_pool(name="psC", bufs=1, space=bass.MemorySpace.PSUM) as psC,
                tc.tile_pool(name="eT", bufs=2) as ep,
                tc.tile_pool(name="rc", bufs=2) as rp,
            ):
                for qb in range(QB):
                    qsl = slice(qb * QBS, (qb + 1) * QBS)
                    for pr in range(4):
                        g = pr // 2
                        ctxp = [psC.tile([65, QBS], f32, name=f"ctx{h2}", tag=f"ctx{h2}") for h2 in range(2)]
                        for grp in range(NGRP):
                            sc2 = [psS.tile([128, KGRP * QBS], f32, name=f"sc{i}", tag="sc") for i in range(2)]
                            for j in range(KGRP):
                                kc = grp * KGRP + j
                                for h2 in range(2):
                                    hs = slice(h2 * 64, (h2 + 1) * 64)
                                    nc.tensor.matmul(
                                        sc2[h2][:, j * QBS : (j + 1) * QBS],
                                        kT2[g][hs, kc * 128 : (kc + 1) * 128],
                                        qT2[pr][hs, qsl],
                                        start=True,
                                        stop=True,
                                        tile_position=(h2 * 64, 0),
                                    )
                            eT2 = [ep.tile([128, KGRP * QBS], f16, name=f"eT{h2}", tag=f"eT{h2}") for h2 in range(2)]
                            for h2 in range(2):
                                nc.scalar.activation(
                                    eT2[h2][:], sc2[h2][:], ACTF.Exp, scale=SCALE
                                )
                            for j in range(KGRP):
                                kc = grp * KGRP + j
                                vsl = slice(kc * 130 + g * 65, kc * 130 + (g + 1) * 65)
                                for h2 in range(2):
                                    nc.tensor.matmul(
                                        ctxp[h2][:],
                                        v_sb[:, vsl],
                                        eT2[h2][:, j * QBS : (j + 1) * QBS],
                                        start=(kc == 0),
                                        stop=(kc == SC - 1),
                                    )
                        for h2 in range(2):
                            recip = rp.tile([1, QBS], f32r, tag="recip")
                            with nc.allow_low_precision(reason="f32r is 4-byte"):
                                nc.vector.reciprocal(recip[:], ctxp[h2][64:65, :])
                            bc = rp.tile([64, QBS], f32r, tag="bc")
                            nc.gpsimd.partition_broadcast(bc[:], recip[:])
                            hs = slice(h2 * 64, (h2 + 1) * 64)
                            nc.vector.tensor_tensor(
                                out=ctxT2[pr][hs, qsl],
                                in0=ctxp[h2][0:64, :],
                                in1=bc[:],
                                op=ALU.mult,
                            )

            # ---- phase C: out projection (partial sum over local heads) ----
            with (
                tc.tile_pool(name="psO", bufs=2, space=bass.MemorySpace.PSUM) as psO,
                tc.tile_pool(name="stC", bufs=3) as st,
                tc.tile_pool(name="woP", bufs=1) as wop,
            ):
                Wo_sb = [wop.tile([128, D], f16, name=f"wo{p}", tag=f"wo{p}") for p in range(4)]
                for p in range(4):
                    nc.sync.dma_start(Wo_sb[p][:], Wo[p * 128 : (p + 1) * 128, :])
                for qt in range(QT):
                    ops = psO.tile([128, D], f32, tag="out")
                    for p in range(4):
                        for nn in range(4):
                            nc.tensor.matmul(
                                ops[:, nn * 512 : (nn + 1) * 512],
                                ctxT2[p][:, qt * 128 : (qt + 1) * 128],
                                Wo_sb[p][:, nn * 512 : (nn + 1) * 512],
                                start=(p == 0),
                                stop=(p == 3),
                            )
                    osb = st.tile([128, D], f32, tag="osb")
                    nc.any.tensor_copy(osb[:], ops[:])
                    nc.sync.dma_start(y[qt * 128 : (qt + 1) * 128, :], osb[:])

    nc.compile()
    return nc


def _get_nc():
    if "nc" not in _CACHE:
        _CACHE["nc"] = _build_bass()
    return _CACHE["nc"]


def make_in_maps(x, Wq, bq, Wk, bk, Wv, bv, Wo):
    f16 = np.float16
    xTb = [np.ascontiguousarray(x[b].T).astype(f16) for b in range(2)]
    Wq16 = Wq.astype(f16)
    Wk16 = Wk.astype(f16)
    Wv16 = Wv.astype(f16)
    Wo16 = Wo.astype(f16)
    in_maps = []
    for c in range(N_CORES):
        b, sh = divmod(c, 4)
        in_maps.append(
            {
                "xT": xTb[b],
                "Wq": np.ascontiguousarray(Wq16[:, sh * CPS : (sh + 1) * CPS]),
                "Wk": np.ascontiguousarray(Wk16[:, sh * KPS : (sh + 1) * KPS]),
                "Wv": np.ascontiguousarray(Wv16[:, sh * KPS : (sh + 1) * KPS]),
                "Wo": np.ascontiguousarray(Wo16[sh * CPS : (sh + 1) * CPS, :]),
                "bq": np.ascontiguousarray(bq[sh * CPS : (sh + 1) * CPS]),
                "bk": np.ascontiguousarray(bk[sh * KPS : (sh + 1) * KPS]),
                "bv": np.ascontiguousarray(bv[sh * KPS : (sh + 1) * KPS]),
            }
        )
    return in_maps


def kernel(x, Wq, bq, Wk, bk, Wv, bv, Wo, bo):
    from concourse.bass_utils import run_bass_kernel_spmd

    x = np.asarray(x, dtype=np.float32)
    Wq = np.asarray(Wq, dtype=np.float32)
    Wk = np.asarray(Wk, dtype=np.float32)
    Wv = np.asarray(Wv, dtype=np.float32)
    Wo = np.asarray(Wo, dtype=np.float32)
    bq = np.asarray(bq, dtype=np.float32)
    bk = np.asarray(bk, dtype=np.float32)
    bv = np.asarray(bv, dtype=np.float32)
    bo = np.asarray(bo, dtype=np.float32)

    in_maps = make_in_maps(x, Wq, bq, Wk, bk, Wv, bv, Wo)
    nc = _get_nc()
    res = run_bass_kernel_spmd(nc, in_maps, core_ids=list(range(N_CORES)))
    out = np.zeros((2, S, D), dtype=np.float32)
    for c in range(N_CORES):
        b = c // 4
        out[b] += res.results[c]["y"]
    out += bo
    return out


# revision 10
# speedup vs baseline: 1.1637x; 1.0160x over previous
"""GQA kernel for Trainium2, 8 NeuronCores.

Sharding: 2 batches x 4 head-shards. Each core handles one batch and
2 KV groups (= 8 Q heads, 512 of the 2048 head-concat columns).
Per core the out-projection produces a partial [S, D] sum; the host
adds the 4 partials per batch (the "all-reduce after out_proj") + bo.

All matmul operands are fp16 (full-rate PE; f32r runs at half clock).
Score matmuls contract over head_dim=64, so the two heads of a pair
are issued as row-tiles (rows 0-63 / 64-127). Phase B is ACT(exp)-
bound, so the out-projection for a query block is emitted inline right
after its attention finishes, filling the PE's slack. Within a pair,
ctx matmuls trail the score/exp pipeline by one group so the PE never
stalls on the softmax-normalization chain at pair boundaries.

Device-side math per core (b = batch, columns c0 = shard*512):
  qT2[pr] = (x_b @ Wq[:, c0+128pr : +128] + bq).T        [128, S]  (head pair)
  kT2[g]  = ((x_b @ Wk[:, ...] + bk)).T, duplicated on both
            partition halves so either q-head parity can use it  [128, S]
  v       = x_b @ Wv + bv, stored per key-chunk as [64 v_g | 1]  [128, 16*130]
  scT     = kT chunk^T x qT  (keys on partitions)               [128, 512]
  eT      = exp(scT / 8)   (no max subtraction: scores ~ N(0,1))
  ctxT    = [v_g | 1]^T @ eT -> rows 0..63 ctx^T, row 64 = softmax sums
  ctxT'   = ctxT * (1/sums)  (gpsimd partition broadcast + DVE mult)
  y_part  = sum_pr ctxT2'[pr]^T @ Wo[...]                        [S, D]

x is pre-transposed + fp16-cast on the host so no on-device x
transpose is needed.
"""

import sys

sys.path.insert(0, "/opt/trn_rl_repo")

import numpy as np

N_CORES = 8
S = 2048  # sequence length
D = 2048  # d_model
HD = 64  # head dim
HL = 8  # local Q heads per core
GL = 2  # local KV groups per core
CPS = 512  # q/out columns per shard
KPS = 128  # kv columns per shard
SCALE = 1.0 / 8.0  # 1/sqrt(HD)

_CACHE = {}


def _build_bass():
    import concourse.bass as bass
    import concourse.bacc as bacc
    import concourse.mybir as mybir
    import concourse.tile as tile
    from concourse.masks import make_identity

    f32 = mybir.dt.float32
    f32r = mybir.dt.float32r
    f16 = mybir.dt.float16
    ALU = mybir.AluOpType
    ACTF = mybir.ActivationFunctionType

    nc = bacc.Bacc("TRN2", target_bir_lowering=False)

    xT = nc.dram_tensor("xT", [D, S], f16, kind="ExternalInput")
    Wq = nc.dram_tensor("Wq", [D, CPS], f16, kind="ExternalInput")
    Wk = nc.dram_tensor("Wk", [D, KPS], f16, kind="ExternalInput")
    Wv = nc.dram_tensor("Wv", [D, KPS], f16, kind="ExternalInput")
    Wo = nc.dram_tensor("Wo", [CPS, D], f16, kind="ExternalInput")
    bq = nc.dram_tensor("bq", [CPS], f32, kind="ExternalInput")
    bk = nc.dram_tensor("bk", [KPS], f32, kind="ExternalInput")
    bv = nc.dram_tensor("bv", [KPS], f32, kind="ExternalInput")
    y = nc.dram_tensor("y", [S, D], f32, kind="ExternalOutput")

    DC = D // 128  # 16 contraction chunks for projections
    SC = S // 128  # 16 key chunks
    QB = 4  # query blocks of 512 in attention
    QBS = S // QB
    KGRP = 2  # key chunks per exp group (psum tile = 2 banks f32)
    NGRP = SC // KGRP  # exp groups per (head, qb)

    with tile.TileContext(nc) as tc:
        with tc.tile_pool(name="persist", bufs=1) as pp:
            # ---- persistent SBUF tensors (per-partition KB in comments) ----
            qT2 = [pp.tile([128, S], f16, name=f"qT{p}", tag=f"qT{p}") for p in range(4)]  # 16
            kT2 = [pp.tile([128, S], f16, name=f"kT{g}", tag=f"kT{g}") for g in range(GL)]  # 8
            # v with a ones column appended per group: 16 chunks x ([64 v|1] x2)
            v_sb = pp.tile([128, SC * 130], f16, tag="v_sb")  # 4.1
            ctxT2 = [pp.tile([128, S], f16, name=f"ctxT{p}", tag=f"ctxT{p}") for p in range(4)]  # 16
            Wo_sb = [pp.tile([128, D], f16, name=f"wo{p}", tag=f"wo{p}") for p in range(4)]  # 16
            bqs = [pp.tile([128, 1], f32, name=f"bq{t}", tag=f"bq{t}") for t in range(4)]
            bks = pp.tile([128, 1], f32, tag="bks")
            bvs = pp.tile([128, 1], f32, tag="bvs")
            ident = pp.tile([128, 128], f16, tag="ident")
            vones = pp.tile([128, 1], f16, tag="vones")
            ident_f32 = pp.tile([128, 128], f32, tag="ident_f32")

            nc.gpsimd.memset(vones[:], 1.0)
            for k in range(2 * SC):
                nc.vector.tensor_copy(v_sb[:, 64 + 65 * k : 65 + 65 * k], vones[:])
            make_identity(nc, ident_f32[:])
            nc.vector.tensor_copy(ident[:], ident_f32[:])

            for t in range(4):
                nc.sync.dma_start(bqs[t][:], bq[t * 128 : (t + 1) * 128])
            nc.sync.dma_start(bks[:], bk[:])
            nc.sync.dma_start(bvs[:], bv[:])

            # ---- phase A: projections ----
            # All of Wq|Wk|Wv resident: 24 KB/part fp16. One xT pass feeds
            # 6 parallel PSUM accumulators (qT pairs x4, kT, vT).
            with (
                tc.tile_pool(name="wall", bufs=1) as wp,
                tc.tile_pool(name="xpre", bufs=1) as xp,
                tc.tile_pool(name="stA", bufs=8) as st,
                tc.tile_pool(name="psA", bufs=1, space=bass.MemorySpace.PSUM) as psA,
                tc.tile_pool(name="psT", bufs=2, space=bass.MemorySpace.PSUM) as psT,
            ):
                # interleave sq0's x tiles with the weight loads so the
                # first matmul isn't queued behind all 48 weight DMAs
                Wall = [wp.tile([128, 768], f16, name=f"wall{i}", tag=f"wall{i}") for i in range(DC)]
                xt0 = [xp.tile([128, 512], f16, name=f"xt0_{i}", tag=f"xt0_{i}") for i in range(DC)]
                for dc in range(DC):
                    rs = slice(dc * 128, (dc + 1) * 128)
                    nc.sync.dma_start(xt0[dc][:], xT[rs, 0:512])
                    nc.sync.dma_start(Wall[dc][:, 0:512], Wq[rs, :])
                    nc.sync.dma_start(Wall[dc][:, 512:640], Wk[rs, :])
                    nc.sync.dma_start(Wall[dc][:, 640:768], Wv[rs, :])
                for p in range(4):
                    nc.sync.dma_start(Wo_sb[p][:], Wo[p * 128 : (p + 1) * 128, :])

                for sq in range(4):
                    s0 = sq * 512
                    pss = [
                        psA.tile([128, 512], f32, name=f"proj{ct}_{sq}", tag=f"proj{ct}")
                        for ct in range(6)
                    ]
                    for dc in range(DC):
                        if sq == 0:
                            xt = xt0[dc]
                        else:
                            xt = st.tile([128, 512], f16, tag="xt")
                            nc.sync.dma_start(xt[:], xT[dc * 128 : (dc + 1) * 128, s0 : s0 + 512])
                        for ct in range(6):
                            nc.tensor.matmul(
                                pss[ct][:],
                                Wall[dc][:, ct * 128 : (ct + 1) * 128],
                                xt[:],
                                start=(dc == 0),
                                stop=(dc == DC - 1),
                            )
                    sl = slice(s0, s0 + 512)
                    for ct in range(4):
                        nc.vector.tensor_scalar_add(qT2[ct][:, sl], pss[ct][:], bqs[ct][:])
                    for g in range(GL):
                        gs = slice(g * 64, (g + 1) * 64)
                        for half in range(2):
                            hs = slice(half * 64, (half + 1) * 64)
                            nc.vector.tensor_scalar_add(
                                kT2[g][hs, sl], pss[4][gs, :], bks[gs, :]
                            )
                    vt = st.tile([128, 512], f16, tag="vt")
                    nc.vector.tensor_scalar_add(vt[:], pss[5][:], bvs[:])
                    for c4 in range(4):
                        tck = sq * 4 + c4
                        tp = psT.tile([128, 128], f16, tag="vtp")
                        nc.tensor.transpose(tp[:], vt[:, c4 * 128 : (c4 + 1) * 128], ident[:])
                        for g in range(GL):
                            nc.vector.tensor_copy(
                                v_sb[:, tck * 130 + g * 65 : tck * 130 + g * 65 + 64],
                                tp[:, g * 64 : (g + 1) * 64],
                            )

            # ---- phase B + C: attention with inlined out-projection ----
            with (
                tc.tile_pool(name="psS", bufs=2, space=bass.MemorySpace.PSUM) as psS,
                tc.tile_pool(name="psC", bufs=1, space=bass.MemorySpace.PSUM) as psC,
                tc.tile_pool(name="psO", bufs=1, space=bass.MemorySpace.PSUM) as psO,
                tc.tile_pool(name="eT", bufs=3) as ep,
                tc.tile_pool(name="rc", bufs=2) as rp,
                tc.tile_pool(name="stC", bufs=3) as so,
            ):
                for qb in range(QB):
                    qsl = slice(qb * QBS, (qb + 1) * QBS)
                    for pr in range(4):
                        g = pr // 2
                        ctxp = [psC.tile([65, QBS], f32, name=f"ctx{h2}", tag=f"ctx{h2}") for h2 in range(2)]
                        eTs = [None] * NGRP

                        def scores(grp):
                            sc2 = [psS.tile([128, KGRP * QBS], f32, name=f"sc{i}", tag="sc") for i in range(2)]
                            for j in range(KGRP):
                                kc = grp * KGRP + j
                                for h2 in range(2):
                                    hs = slice(h2 * 64, (h2 + 1) * 64)
                                    nc.tensor.matmul(
                                        sc2[h2][:, j * QBS : (j + 1) * QBS],
                                        kT2[g][hs, kc * 128 : (kc + 1) * 128],
                                        qT2[pr][hs, qsl],
                                        start=True,
                                        stop=True,
                                        tile_position=(h2 * 64, 0),
                                    )
                            eT2 = [ep.tile([128, KGRP * QBS], f16, name=f"eT{h2}", tag=f"eT{h2}") for h2 in range(2)]
                            for h2 in range(2):
                                nc.scalar.activation(
                                    eT2[h2][:], sc2[h2][:], ACTF.Exp, scale=SCALE
                                )
                            eTs[grp] = eT2

                        def ctx(grp):
                            eT2 = eTs[grp]
                            for j in range(KGRP):
                                kc = grp * KGRP + j
                                vsl = slice(kc * 130 + g * 65, kc * 130 + (g + 1) * 65)
                                for h2 in range(2):
                                    nc.tensor.matmul(
                                        ctxp[h2][:],
                                        v_sb[:, vsl],
                                        eT2[h2][:, j * QBS : (j + 1) * QBS],
                                        start=(kc == 0),
                                        stop=(kc == SC - 1),
                                    )

                        # ctx trails scores by one group: during the previous
                        # pair's normalization the PE runs this pair's scores.
                        scores(0)
                        for grp in range(1, NGRP):
                            scores(grp)
                            ctx(grp - 1)
                        ctx(NGRP - 1)

                        for h2 in range(2):
                            recip = rp.tile([1, QBS], f32r, tag="recip")
                            with nc.allow_low_precision(reason="f32r is 4-byte"):
                                nc.vector.reciprocal(recip[:], ctxp[h2][64:65, :])
                            bc = rp.tile([64, QBS], f32r, tag="bc")
                            nc.gpsimd.partition_broadcast(bc[:], recip[:])
                            hs = slice(h2 * 64, (h2 + 1) * 64)
                            nc.vector.tensor_tensor(
                                out=ctxT2[pr][hs, qsl],
                                in0=ctxp[h2][0:64, :],
                                in1=bc[:],
                                op=ALU.mult,
                            )

                    # out-projection for this query block (4 row-tiles of 128)
                    for qt4 in range(4):
                        qt = qb * 4 + qt4
                        for half in range(2):
                            ops = psO.tile([128, D // 2], f32, tag="out")
                            for p in range(4):
                                for nn in range(2):
                                    n0 = half * 1024 + nn * 512
                                    nc.tensor.matmul(
                                        ops[:, nn * 512 : (nn + 1) * 512],
                                        ctxT2[p][:, qt * 128 : (qt + 1) * 128],
                                        Wo_sb[p][:, n0 : n0 + 512],
                                        start=(p == 0),
                                        stop=(p == 3),
                                    )
                            osb = so.tile([128, D // 2], f32, name=f"osb{half}", tag="osb")
                            nc.any.tensor_copy(osb[:], ops[:])
                            nc.sync.dma_start(
                                y[qt * 128 : (qt + 1) * 128, half * 1024 : (half + 1) * 1024],
                                osb[:],
                            )

    nc.compile()
    return nc


def _get_nc():
    if "nc" not in _CACHE:
        _CACHE["nc"] = _build_bass()
    return _CACHE["nc"]


def make_in_maps(x, Wq, bq, Wk, bk, Wv, bv, Wo):
    f16 = np.float16
    xTb = [np.ascontiguousarray(x[b].T).astype(f16) for b in range(2)]
    Wq16 = Wq.astype(f16)
    Wk16 = Wk.astype(f16)
    Wv16 = Wv.astype(f16)
    Wo16 = Wo.astype(f16)
    in_maps = []
    for c in range(N_CORES):
        b, sh = divmod(c, 4)
        in_maps.append(
            {
                "xT": xTb[b],
                "Wq": np.ascontiguousarray(Wq16[:, sh * CPS : (sh + 1) * CPS]),
                "Wk": np.ascontiguousarray(Wk16[:, sh * KPS : (sh + 1) * KPS]),
                "Wv": np.ascontiguousarray(Wv16[:, sh * KPS : (sh + 1) * KPS]),
                "Wo": np.ascontiguousarray(Wo16[sh * CPS : (sh + 1) * CPS, :]),
                "bq": np.ascontiguousarray(bq[sh * CPS : (sh + 1) * CPS]),
                "bk": np.ascontiguousarray(bk[sh * KPS : (sh + 1) * KPS]),
                "bv": np.ascontiguousarray(bv[sh * KPS : (sh + 1) * KPS]),
            }
        )
    return in_maps


def kernel(x, Wq, bq, Wk, bk, Wv, bv, Wo, bo):
    from concourse.bass_utils import run_bass_kernel_spmd

    x = np.asarray(x, dtype=np.float32)
    Wq = np.asarray(Wq, dtype=np.float32)
    Wk = np.asarray(Wk, dtype=np.float32)
    Wv = np.asarray(Wv, dtype=np.float32)
    Wo = np.asarray(Wo, dtype=np.float32)
    bq = np.asarray(bq, dtype=np.float32)
    bk = np.asarray(bk, dtype=np.float32)
    bv = np.asarray(bv, dtype=np.float32)
    bo = np.asarray(bo, dtype=np.float32)

    in_maps = make_in_maps(x, Wq, bq, Wk, bk, Wv, bv, Wo)
    nc = _get_nc()
    res = run_bass_kernel_spmd(nc, in_maps, core_ids=list(range(N_CORES)))
    out = np.zeros((2, S, D), dtype=np.float32)
    for c in range(N_CORES):
        b = c // 4
        out[b] += res.results[c]["y"]
    out += bo
    return out


# revision 13
# speedup vs baseline: 1.1817x; 1.0155x over previous
"""GQA kernel for Trainium2, 8 NeuronCores.

Sharding: 2 batches x 4 head-shards. Each core handles one batch and
2 KV groups (= 8 Q heads, 512 of the 2048 head-concat columns).
Per core the out-projection produces a partial [S, D] sum; the host
adds the 4 partials per batch (the "all-reduce after out_proj") + bo.

All matmul operands are fp16 (full-rate PE; f32r runs at half clock).
Phase B is ACT(exp)-bound, so everything else is software-pipelined
around the continuous score->exp stream:

  prefix:  K/V projections (+ v transposes) for all of S, plus the
           q-projection for the first head pair.
  steady state, one "group" (2 key chunks x 512 queries) per step:
    - ctx matmuls for the group LAG steps behind (their exps are done)
    - 2 q-projection matmuls for the NEXT head pair
    - 2 out-projection quarter-tiles of the PREVIOUS query block
    - score matmuls (two heads as concurrent row-tiles) + 2 exps
  The softmax normalization (DVE reciprocal of the ones-row sums,
  gpsimd partition broadcast, DVE multiply) runs LAG groups behind the
  scores, so its latency hides under the running exp stream.

Device-side math per core (b = batch, columns c0 = shard*512):
  qT2[pr] = (x_b @ Wq[:, c0+128pr : +128] + bq).T        [128, S]  (head pair)
  kT2[g]  = ((x_b @ Wk[:, ...] + bk)).T, duplicated on both halves [128, S]
  v       = x_b @ Wv + bv, stored per key-chunk as [64 v_g | 1]  [128, 16*130]
  scT     = kT chunk^T x qT  (keys on partitions)               [128, 512]
  eT      = exp(scT / 8)   (no max subtraction: scores ~ N(0,1))
  ctxT    = [v_g | 1]^T @ eT -> rows 0..63 ctx^T, row 64 = softmax sums
  ctxT'   = ctxT * (1/sums)
  y_part  = sum_pr ctxT2'[pr]^T @ Wo[...]                        [S, D]

x is pre-transposed + fp16-cast on the host.
"""

import sys

sys.path.insert(0, "/opt/trn_rl_repo")

import numpy as np

N_CORES = 8
S = 2048  # sequence length
D = 2048  # d_model
HD = 64  # head dim
GL = 2  # local KV groups per core
CPS = 512  # q/out columns per shard
KPS = 128  # kv columns per shard
SCALE = 1.0 / 8.0  # 1/sqrt(HD)

_CACHE = {}


def _build_bass():
    import concourse.bass as bass
    import concourse.bacc as bacc
    import concourse.mybir as mybir
    import concourse.tile as tile
    from concourse.masks import make_identity

    f32 = mybir.dt.float32
    f32r = mybir.dt.float32r
    f16 = mybir.dt.float16
    ALU = mybir.AluOpType
    ACTF = mybir.ActivationFunctionType

    nc = bacc.Bacc("TRN2", target_bir_lowering=False)

    xT = nc.dram_tensor("xT", [D, S], f16, kind="ExternalInput")
    Wq = nc.dram_tensor("Wq", [D, CPS], f16, kind="ExternalInput")
    Wk = nc.dram_tensor("Wk", [D, KPS], f16, kind="ExternalInput")
    Wv = nc.dram_tensor("Wv", [D, KPS], f16, kind="ExternalInput")
    Wo = nc.dram_tensor("Wo", [CPS, D], f16, kind="ExternalInput")
    bq = nc.dram_tensor("bq", [CPS], f32, kind="ExternalInput")
    bk = nc.dram_tensor("bk", [KPS], f32, kind="ExternalInput")
    bv = nc.dram_tensor("bv", [KPS], f32, kind="ExternalInput")
    y = nc.dram_tensor("y", [S, D], f32, kind="ExternalOutput")

    DC = D // 128  # 16 contraction chunks for projections
    SC = S // 128  # 16 key chunks
    QB = 4  # query blocks of 512 in attention
    QBS = S // QB
    KGRP = 2  # key chunks per exp group (psum tile = 2 banks f32)
    NGRP = SC // KGRP  # groups per (pair, qb)
    NPAIR = 16  # (qb, pr) pairs
    LAG = 6  # ctx trails scores by this many groups (global pipeline)

    with tile.TileContext(nc) as tc:
        with (
            tc.tile_pool(name="persist", bufs=1) as pp,
            tc.tile_pool(name="xq", bufs=2) as xqp,
            tc.tile_pool(name="psQ", bufs=1, space=bass.MemorySpace.PSUM) as psQ,
        ):
            # ---- persistent SBUF tensors ----
            qT2 = [pp.tile([128, S], f16, name=f"qT{p}", tag=f"qT{p}") for p in range(4)]
            kT2 = [pp.tile([128, S], f16, name=f"kT{g}", tag=f"kT{g}") for g in range(GL)]
            v_sb = pp.tile([128, SC * 130], f16, tag="v_sb")
            ctxT2 = [pp.tile([128, S], f16, name=f"ctxT{p}", tag=f"ctxT{p}") for p in range(4)]
            Wo_sb = [pp.tile([128, D], f16, name=f"wo{p}", tag=f"wo{p}") for p in range(4)]
            Wall = [pp.tile([128, 768], f16, name=f"wall{i}", tag=f"wall{i}") for i in range(DC)]
            bqs = [pp.tile([128, 1], f32, name=f"bq{t}", tag=f"bq{t}") for t in range(4)]
            bks = pp.tile([128, 1], f32, tag="bks")
            bvs = pp.tile([128, 1], f32, tag="bvs")
            ident = pp.tile([128, 128], f16, tag="ident")
            vones = pp.tile([128, 1], f16, tag="vones")
            ident_f32 = pp.tile([128, 128], f32, tag="ident_f32")

            nc.gpsimd.memset(vones[:], 1.0)
            for k in range(2 * SC):
                nc.vector.tensor_copy(v_sb[:, 64 + 65 * k : 65 + 65 * k], vones[:])
            make_identity(nc, ident_f32[:])
            nc.vector.tensor_copy(ident[:], ident_f32[:])

            for t in range(4):
                nc.sync.dma_start(bqs[t][:], bq[t * 128 : (t + 1) * 128])
            nc.sync.dma_start(bks[:], bk[:])
            nc.sync.dma_start(bvs[:], bv[:])

            # x tiles for q-projections, cached per (qb, dc)
            xq_tiles = {}

            def get_xq(qb, dc):
                key = (qb, dc)
                if key not in xq_tiles:
                    t = xqp.tile([128, 512], f16, name=f"xq{dc}", tag=f"xq{dc}")
                    nc.sync.dma_start(
                        t[:], xT[dc * 128 : (dc + 1) * 128, qb * 512 : (qb + 1) * 512]
                    )
                    xq_tiles[key] = t
                return xq_tiles[key]

            # q-projection accumulators, one live at a time
            qacc = {}

            def qproj_mms(pair, dcs):
                qb, pr = divmod(pair, 4)
                if pair not in qacc:
                    qacc[pair] = psQ.tile([128, 512], f32, name=f"qa{pair}", tag="qacc")
                for dc in dcs:
                    nc.tensor.matmul(
                        qacc[pair][:],
                        Wall[dc][:, pr * 128 : (pr + 1) * 128],
                        get_xq(qb, dc)[:],
                        start=(dc == 0),
                        stop=(dc == DC - 1),
                    )

            def qproj_finish(pair):
                qb, pr = divmod(pair, 4)
                nc.vector.tensor_scalar_add(
                    qT2[pr][:, qb * 512 : (qb + 1) * 512], qacc[pair][:], bqs[pr][:]
                )
                del qacc[pair]

            # ---- prefix: weight loads, K/V projections, qproj(pair 0) ----
            with (
                tc.tile_pool(name="stA", bufs=8) as st,
                tc.tile_pool(name="psKV", bufs=1, space=bass.MemorySpace.PSUM) as psKV,
                tc.tile_pool(name="psT", bufs=2, space=bass.MemorySpace.PSUM) as psT,
            ):
                # first-needed DMAs first: x tiles for sq0 + K/V weights
                for dc in range(DC):
                    rs = slice(dc * 128, (dc + 1) * 128)
                    get_xq(0, dc)
                    nc.sync.dma_start(Wall[dc][:, 512:640], Wk[rs, :])
                    nc.sync.dma_start(Wall[dc][:, 640:768], Wv[rs, :])
                for dc in range(DC):
                    rs = slice(dc * 128, (dc + 1) * 128)
                    nc.sync.dma_start(Wall[dc][:, 0:512], Wq[rs, :])
                for p in range(4):
                    nc.sync.dma_start(Wo_sb[p][:], Wo[p * 128 : (p + 1) * 128, :])

                for sq in range(4):
                    s0 = sq * 512
                    kps = psKV.tile([128, 512], f32, name=f"kp{sq}", tag="kp")
                    vps = psKV.tile([128, 512], f32, name=f"vp{sq}", tag="vp")
                    for dc in range(DC):
                        if sq == 0:
                            xt = get_xq(0, dc)
                        else:
                            xt = st.tile([128, 512], f16, tag="xt")
                            nc.sync.dma_start(
                                xt[:], xT[dc * 128 : (dc + 1) * 128, s0 : s0 + 512]
                            )
                        nc.tensor.matmul(
                            kps[:], Wall[dc][:, 512:640], xt[:],
                            start=(dc == 0), stop=(dc == DC - 1),
                        )
                        nc.tensor.matmul(
                            vps[:], Wall[dc][:, 640:768], xt[:],
                            start=(dc == 0), stop=(dc == DC - 1),
                        )
                        if sq == 0:
                            qproj_mms(0, [dc])
                    sl = slice(s0, s0 + 512)
                    for g in range(GL):
                        gs = slice(g * 64, (g + 1) * 64)
                        for half in range(2):
                            hs = slice(half * 64, (half + 1) * 64)
                            nc.vector.tensor_scalar_add(
                                kT2[g][hs, sl], kps[gs, :], bks[gs, :]
                            )
                    vt = st.tile([128, 512], f16, tag="vt")
                    nc.vector.tensor_scalar_add(vt[:], vps[:], bvs[:])
                    for c4 in range(4):
                        tck = sq * 4 + c4
                        tp = psT.tile([128, 128], f16, tag="vtp")
                        nc.tensor.transpose(tp[:], vt[:, c4 * 128 : (c4 + 1) * 128], ident[:])
                        for g in range(GL):
                            nc.vector.tensor_copy(
                                v_sb[:, tck * 130 + g * 65 : tck * 130 + g * 65 + 64],
                                tp[:, g * 64 : (g + 1) * 64],
                            )
                qproj_finish(0)

            # ---- phase B: pipelined attention + spread out-projection ----
            with (
                tc.tile_pool(name="psS", bufs=2, space=bass.MemorySpace.PSUM) as psS,
                tc.tile_pool(name="psC", bufs=1, space=bass.MemorySpace.PSUM) as psC,
                tc.tile_pool(name="psO", bufs=1, space=bass.MemorySpace.PSUM) as psO,
                tc.tile_pool(name="eT", bufs=LAG + 2) as ep,
                tc.tile_pool(name="rc", bufs=2) as rp,
                tc.tile_pool(name="stC", bufs=3) as so,
            ):
                ctx_tiles = {}  # pair -> [ctx psum tile per head]
                eT_store = {}  # global group G -> [eT tile per head]

                def scores(G):
                    pair, grp = divmod(G, NGRP)
                    qb, pr = divmod(pair, 4)
                    g = pr // 2
                    qsl = slice(qb * QBS, (qb + 1) * QBS)
                    sc2 = [psS.tile([128, KGRP * QBS], f32, name=f"sc{i}", tag="sc") for i in range(2)]
                    for j in range(KGRP):
                        kc = grp * KGRP + j
                        for h2 in range(2):
                            hs = slice(h2 * 64, (h2 + 1) * 64)
                            nc.tensor.matmul(
                                sc2[h2][:, j * QBS : (j + 1) * QBS],
                                kT2[g][hs, kc * 128 : (kc + 1) * 128],
                                qT2[pr][hs, qsl],
                                start=True,
                                stop=True,
                                tile_position=(h2 * 64, 0),
                            )
                    eT2 = [ep.tile([128, KGRP * QBS], f16, name=f"eT{h2}", tag=f"eT{h2}") for h2 in range(2)]
                    for h2 in range(2):
                        nc.scalar.activation(eT2[h2][:], sc2[h2][:], ACTF.Exp, scale=SCALE)
                    eT_store[G] = eT2

                def ctx(G):
                    pair, grp = divmod(G, NGRP)
                    g = (pair % 4) // 2
                    if pair not in ctx_tiles:
                        ctx_tiles[pair] = [
                            psC.tile([65, QBS], f32, name=f"ctx{h2}", tag=f"ctx{h2}")
                            for h2 in range(2)
                        ]
                    eT2 = eT_store.pop(G)
                    for j in range(KGRP):
                        kc = grp * KGRP + j
                        vsl = slice(kc * 130 + g * 65, kc * 130 + (g + 1) * 65)
                        for h2 in range(2):
                            nc.tensor.matmul(
                                ctx_tiles[pair][h2][:],
                                v_sb[:, vsl],
                                eT2[h2][:, j * QBS : (j + 1) * QBS],
                                start=(kc == 0),
                                stop=(kc == SC - 1),
                            )
                    if grp == NGRP - 1:
                        normalize(pair)

                def normalize(pair):
                    qb, pr = divmod(pair, 4)
                    qsl = slice(qb * QBS, (qb + 1) * QBS)
                    ctxp = ctx_tiles.pop(pair)
                    for h2 in range(2):
                        recip = rp.tile([1, QBS], f32r, tag="recip")
                        with nc.allow_low_precision(reason="f32r is 4-byte"):
                            nc.vector.reciprocal(recip[:], ctxp[h2][64:65, :])
                        bc = rp.tile([64, QBS], f32r, tag="bc")
                        nc.gpsimd.partition_broadcast(bc[:], recip[:])
                        hs = slice(h2 * 64, (h2 + 1) * 64)
                        nc.vector.tensor_tensor(
                            out=ctxT2[pr][hs, qsl],
                            in0=ctxp[h2][0:64, :],
                            in1=bc[:],
                            op=ALU.mult,
                        )

                def outproj_quarter(qb, qi):
                    qt = qb * 4 + qi // 4
                    qtr = qi % 4
                    ops = psO.tile([128, 512], f32, tag="out")
                    for p in range(4):
                        nc.tensor.matmul(
                            ops[:],
                            ctxT2[p][:, qt * 128 : (qt + 1) * 128],
                            Wo_sb[p][:, qtr * 512 : (qtr + 1) * 512],
                            start=(p == 0),
                            stop=(p == 3),
                        )
                    osb = so.tile([128, 512], f32, tag="osb")
                    nc.vector.tensor_copy(osb[:], ops[:])
                    nc.sync.dma_start(
                        y[qt * 128 : (qt + 1) * 128, qtr * 512 : (qtr + 1) * 512],
                        osb[:],
                    )

                NG = NPAIR * NGRP  # 128 global groups
                for G in range(NG):
                    pair, grp = divmod(G, NGRP)
                    if G - LAG >= 0:
                        ctx(G - LAG)
                    # q-projection for the next pair, 2 dc chunks per group
                    if pair + 1 < NPAIR:
                        nqb = (pair + 1) // 4
                        if (pair + 1) % 4 == 0 and grp < NGRP - 1:
                            # crossing into a new query block: prefetch the x
                            # tiles one group ahead of their matmuls
                            get_xq(nqb, 2 * grp + 2)
                            get_xq(nqb, 2 * grp + 3)
                        if pair % 4 == 2 and pair < 12 and grp == NGRP - 1:
                            get_xq(pair // 4 + 1, 0)
                            get_xq(pair // 4 + 1, 1)
                        qproj_mms(pair + 1, [2 * grp, 2 * grp + 1])
                        if grp == NGRP - 1:
                            qproj_finish(pair + 1)
                    # out-projection of the previous query block, 2 quarters
                    # per group; spread over the second pair of this block so
                    # the previous block's last normalize is already done
                    if pair % 4 == 1 and pair >= 5:
                        outproj_quarter(pair // 4 - 1, 2 * grp)
                        outproj_quarter(pair // 4 - 1, 2 * grp + 1)
                    scores(G)
                # drain: remaining ctx groups, then last block's out-projection
                for G in range(NG - LAG, NG):
                    ctx(G)
                for qi in range(16):
                    outproj_quarter(3, qi)

    nc.compile()
    return nc


def _get_nc():
    if "nc" not in _CACHE:
        _CACHE["nc"] = _build_bass()
    return _CACHE["nc"]


def make_in_maps(x, Wq, bq, Wk, bk, Wv, bv, Wo):
    f16 = np.float16
    xTb = [np.ascontiguousarray(x[b].T).astype(f16) for b in range(2)]
    Wq16 = Wq.astype(f16)
    Wk16 = Wk.astype(f16)
    Wv16 = Wv.astype(f16)
    Wo16 = Wo.astype(f16)
    in_maps = []
    for c in range(N_CORES):
        b, sh = divmod(c, 4)
        in_maps.append(
            {
                "xT": xTb[b],
                "Wq": np.ascontiguousarray(Wq16[:, sh * CPS : (sh + 1) * CPS]),
                "Wk": np.ascontiguousarray(Wk16[:, sh * KPS : (sh + 1) * KPS]),
                "Wv": np.ascontiguousarray(Wv16[:, sh * KPS : (sh + 1) * KPS]),
                "Wo": np.ascontiguousarray(Wo16[sh * CPS : (sh + 1) * CPS, :]),
                "bq": np.ascontiguousarray(bq[sh * CPS : (sh + 1) * CPS]),
                "bk": np.ascontiguousarray(bk[sh * KPS : (sh + 1) * KPS]),
                "bv": np.ascontiguousarray(bv[sh * KPS : (sh + 1) * KPS]),
            }
        )
    return in_maps


def kernel(x, Wq, bq, Wk, bk, Wv, bv, Wo, bo):
    from concourse.bass_utils import run_bass_kernel_spmd

    x = np.asarray(x, dtype=np.float32)
    Wq = np.asarray(Wq, dtype=np.float32)
    Wk = np.asarray(Wk, dtype=np.float32)
    Wv = np.asarray(Wv, dtype=np.float32)
    Wo = np.asarray(Wo, dtype=np.float32)
    bq = np.asarray(bq, dtype=np.float32)
    bk = np.asarray(bk, dtype=np.float32)
    bv = np.asarray(bv, dtype=np.float32)
    bo = np.asarray(bo, dtype=np.float32)

    in_maps = make_in_maps(x, Wq, bq, Wk, bk, Wv, bv, Wo)
    nc = _get_nc()
    res = run_bass_kernel_spmd(nc, in_maps, core_ids=list(range(N_CORES)))
    out = np.zeros((2, S, D), dtype=np.float32)
    for c in range(N_CORES):
        b = c // 4
        out[b] += res.results[c]["y"]
    out += bo
    return out


# revision 15
# speedup vs baseline: 1.3955x; 1.1809x over previous
"""GQA kernel for Trainium2, 8 NeuronCores.

Sharding: 2 batches x 4 head-shards. Each core handles one batch and
2 KV groups (= 8 Q heads, 512 of the 2048 head-concat columns).
Per core the out-projection produces a partial [S, D] sum; the host
adds the 4 partials per batch (the "all-reduce after out_proj") + bo.

All matmul operands are fp16 (full-rate PE; f32r runs at half clock).
xT is SBUF-resident (64 KB/partition) so no projection ever waits on
DMA. Phase B is ACT(exp)-bound and everything else is software-
pipelined around the continuous score->exp stream:

  prefix:  K/V projections (+ v transposes) for all of S, plus the
           q-projection for the first head pair.
  steady state, one "group" (2 key chunks x 512 queries) per step:
    - ctx matmuls for the group LAG steps behind (their exps are done)
    - 2 q-projection matmuls for the NEXT head pair
    - out-projection quarter-tiles of an already-finished query block
    - score matmuls (two heads as concurrent row-tiles) + 2 exps
  When a pair's last ctx group lands, its ctx PSUM is copied to SBUF
  staging at once (freeing the PSUM banks for the next pair) and the
  softmax normalization (DVE reciprocal of the ones-row sums, gpsimd
  partition broadcast, DVE multiply) runs off the staging copy.

Device-side math per core (b = batch, columns c0 = shard*512):
  qT2[pr] = (x_b @ Wq[:, c0+128pr : +128] + bq).T        [128, S]  (head pair)
  kT2[g]  = ((x_b @ Wk[:, ...] + bk)).T, duplicated on both halves [128, S]
  v       = x_b @ Wv + bv, stored per key-chunk as [64 v_g | 1]  [128, 16*130]
  scT     = kT chunk^T x qT  (keys on partitions)               [128, 512]
  eT      = exp(scT / 8)   (no max subtraction: scores ~ N(0,1))
  ctxT    = [v_g | 1]^T @ eT -> rows 0..63 ctx^T, row 64 = softmax sums
  ctxT'   = ctxT * (1/sums)
  y_part  = sum_pr ctxT2'[pr]^T @ Wo[...]                        [S, D]

x is pre-transposed + fp16-cast on the host.
"""

import sys

sys.path.insert(0, "/opt/trn_rl_repo")

import numpy as np

N_CORES = 8
S = 2048  # sequence length
D = 2048  # d_model
HD = 64  # head dim
GL = 2  # local KV groups per core
CPS = 512  # q/out columns per shard
KPS = 128  # kv columns per shard
SCALE = 1.0 / 8.0  # 1/sqrt(HD)

_CACHE = {}


def _build_bass():
    import concourse.bass as bass
    import concourse.bacc as bacc
    import concourse.mybir as mybir
    import concourse.tile as tile
    from concourse.masks import make_identity

    f32 = mybir.dt.float32
    f32r = mybir.dt.float32r
    f16 = mybir.dt.float16
    ALU = mybir.AluOpType
    ACTF = mybir.ActivationFunctionType

    nc = bacc.Bacc("TRN2", target_bir_lowering=False)

    xT = nc.dram_tensor("xT", [D, S], f16, kind="ExternalInput")
    Wq = nc.dram_tensor("Wq", [D, CPS], f16, kind="ExternalInput")
    Wk = nc.dram_tensor("Wk", [D, KPS], f16, kind="ExternalInput")
    Wv = nc.dram_tensor("Wv", [D, KPS], f16, kind="ExternalInput")
    Wo = nc.dram_tensor("Wo", [CPS, D], f16, kind="ExternalInput")
    bq = nc.dram_tensor("bq", [CPS], f32, kind="ExternalInput")
    bk = nc.dram_tensor("bk", [KPS], f32, kind="ExternalInput")
    bv = nc.dram_tensor("bv", [KPS], f32, kind="ExternalInput")
    y = nc.dram_tensor("y", [S, D], f32, kind="ExternalOutput")

    DC = D // 128  # 16 contraction chunks for projections
    SC = S // 128  # 16 key chunks
    QB = 4  # query blocks of 512 in attention
    QBS = S // QB
    KGRP = 2  # key chunks per exp group (psum tile = 2 banks f32)
    NGRP = SC // KGRP  # groups per (pair, qb)
    NPAIR = 16  # (qb, pr) pairs
    LAG = 6  # ctx trails scores by this many groups (global pipeline)

    with tile.TileContext(nc) as tc:
        with (
            tc.tile_pool(name="persist", bufs=1) as pp,
            tc.tile_pool(name="psQ", bufs=1, space=bass.MemorySpace.PSUM) as psQ,
        ):
            # ---- persistent SBUF tensors (per-partition KB) ----
            xTs = [pp.tile([128, S], f16, name=f"xT{dc}", tag=f"xT{dc}") for dc in range(DC)]  # 64
            qT2 = [pp.tile([128, S], f16, name=f"qT{p}", tag=f"qT{p}") for p in range(4)]  # 16
            kT2 = [pp.tile([128, S], f16, name=f"kT{g}", tag=f"kT{g}") for g in range(GL)]  # 8
            v_sb = pp.tile([128, SC * 130], f16, tag="v_sb")  # 4.1
            ctxT2 = [pp.tile([128, S], f16, name=f"ctxT{p}", tag=f"ctxT{p}") for p in range(4)]  # 16
            Wo_sb = [pp.tile([128, D], f16, name=f"wo{p}", tag=f"wo{p}") for p in range(4)]  # 16
            Wq_sb = [pp.tile([128, CPS], f16, name=f"wq{i}", tag=f"wq{i}") for i in range(DC)]  # 16
            bqs = [pp.tile([128, 1], f32, name=f"bq{t}", tag=f"bq{t}") for t in range(4)]
            bks = pp.tile([128, 1], f32, tag="bks")
            bvs = pp.tile([128, 1], f32, tag="bvs")
            ident = pp.tile([128, 128], f16, tag="ident")
            vones = pp.tile([128, 1], f16, tag="vones")
            ident_f32 = pp.tile([128, 128], f32, tag="ident_f32")

            nc.gpsimd.memset(vones[:], 1.0)
            for k in range(2 * SC):
                nc.vector.tensor_copy(v_sb[:, 64 + 65 * k : 65 + 65 * k], vones[:])
            make_identity(nc, ident_f32[:])
            nc.vector.tensor_copy(ident[:], ident_f32[:])

            for t in range(4):
                nc.sync.dma_start(bqs[t][:], bq[t * 128 : (t + 1) * 128])
            nc.sync.dma_start(bks[:], bk[:])
            nc.sync.dma_start(bvs[:], bv[:])

            # q-projection accumulators, one live at a time
            qacc = {}

            def qproj_mms(pair, dcs):
                qb, pr = divmod(pair, 4)
                if pair not in qacc:
                    qacc[pair] = psQ.tile([128, 512], f32, name=f"qa{pair}", tag="qacc")
                for dc in dcs:
                    nc.tensor.matmul(
                        qacc[pair][:],
                        Wq_sb[dc][:, pr * 128 : (pr + 1) * 128],
                        xTs[dc][:, qb * 512 : (qb + 1) * 512],
                        start=(dc == 0),
                        stop=(dc == DC - 1),
                    )

            def qproj_finish(pair):
                qb, pr = divmod(pair, 4)
                nc.vector.tensor_scalar_add(
                    qT2[pr][:, qb * 512 : (qb + 1) * 512], qacc[pair][:], bqs[pr][:]
                )
                del qacc[pair]

            # ---- prefix: weight + x loads, K/V projections, qproj(pair 0) ----
            with (
                tc.tile_pool(name="wkv", bufs=1) as wkvp,
                tc.tile_pool(name="stA", bufs=6) as st,
                tc.tile_pool(name="psKV", bufs=1, space=bass.MemorySpace.PSUM) as psKV,
                tc.tile_pool(name="psT", bufs=2, space=bass.MemorySpace.PSUM) as psT,
            ):
                Wkv = [wkvp.tile([128, 256], f16, name=f"wkv{i}", tag=f"wkv{i}") for i in range(DC)]
                for dc in range(DC):
                    rs = slice(dc * 128, (dc + 1) * 128)
                    nc.sync.dma_start(xTs[dc][:], xT[rs, :])
                    nc.sync.dma_start(Wkv[dc][:, 0:128], Wk[rs, :])
                    nc.sync.dma_start(Wkv[dc][:, 128:256], Wv[rs, :])
                for dc in range(DC):
                    nc.sync.dma_start(Wq_sb[dc][:], Wq[dc * 128 : (dc + 1) * 128, :])
                for p in range(4):
                    nc.sync.dma_start(Wo_sb[p][:], Wo[p * 128 : (p + 1) * 128, :])

                for sq in range(4):
                    s0 = sq * 512
                    ssl = slice(s0, s0 + 512)
                    kps = psKV.tile([128, 512], f32, name=f"kp{sq}", tag="kp")
                    vps = psKV.tile([128, 512], f32, name=f"vp{sq}", tag="vp")
                    for dc in range(DC):
                        nc.tensor.matmul(
                            kps[:], Wkv[dc][:, 0:128], xTs[dc][:, ssl],
                            start=(dc == 0), stop=(dc == DC - 1),
                        )
                        nc.tensor.matmul(
                            vps[:], Wkv[dc][:, 128:256], xTs[dc][:, ssl],
                            start=(dc == 0), stop=(dc == DC - 1),
                        )
                        if sq == 0:
                            qproj_mms(0, [dc])
                    for g in range(GL):
                        gs = slice(g * 64, (g + 1) * 64)
                        for half in range(2):
                            hs = slice(half * 64, (half + 1) * 64)
                            nc.vector.tensor_scalar_add(
                                kT2[g][hs, ssl], kps[gs, :], bks[gs, :]
                            )
                    vt = st.tile([128, 512], f16, tag="vt")
                    nc.vector.tensor_scalar_add(vt[:], vps[:], bvs[:])
                    for c4 in range(4):
                        tck = sq * 4 + c4
                        tp = psT.tile([128, 128], f16, tag="vtp")
                        nc.tensor.transpose(tp[:], vt[:, c4 * 128 : (c4 + 1) * 128], ident[:])
                        for g in range(GL):
                            nc.vector.tensor_copy(
                                v_sb[:, tck * 130 + g * 65 : tck * 130 + g * 65 + 64],
                                tp[:, g * 64 : (g + 1) * 64],
                            )
                qproj_finish(0)

            # ---- phase B: pipelined attention + spread out-projection ----
            with (
                tc.tile_pool(name="psS", bufs=2, space=bass.MemorySpace.PSUM) as psS,
                tc.tile_pool(name="psC", bufs=1, space=bass.MemorySpace.PSUM) as psC,
                tc.tile_pool(name="psO", bufs=1, space=bass.MemorySpace.PSUM) as psO,
                tc.tile_pool(name="eT", bufs=LAG + 1) as ep,
                tc.tile_pool(name="stage", bufs=2) as sg,
                tc.tile_pool(name="rc", bufs=2) as rp,
                tc.tile_pool(name="stC", bufs=3) as so,
            ):
                ctx_tiles = {}  # pair -> [ctx psum tile per head]
                eT_store = {}  # global group G -> [eT tile per head]

                def scores(G):
                    pair, grp = divmod(G, NGRP)
                    qb, pr = divmod(pair, 4)
                    g = pr // 2
                    qsl = slice(qb * QBS, (qb + 1) * QBS)
                    sc2 = [psS.tile([128, KGRP * QBS], f32, name=f"sc{i}", tag="sc") for i in range(2)]
                    for j in range(KGRP):
                        kc = grp * KGRP + j
                        for h2 in range(2):
                            hs = slice(h2 * 64, (h2 + 1) * 64)
                            nc.tensor.matmul(
                                sc2[h2][:, j * QBS : (j + 1) * QBS],
                                kT2[g][hs, kc * 128 : (kc + 1) * 128],
                                qT2[pr][hs, qsl],
                                start=True,
                                stop=True,
                                tile_position=(h2 * 64, 0),
                            )
                    eT2 = [ep.tile([128, KGRP * QBS], f16, name=f"eT{h2}", tag=f"eT{h2}") for h2 in range(2)]
                    for h2 in range(2):
                        nc.scalar.activation(eT2[h2][:], sc2[h2][:], ACTF.Exp, scale=SCALE)
                    eT_store[G] = eT2

                def ctx(G):
                    pair, grp = divmod(G, NGRP)
                    g = (pair % 4) // 2
                    if pair not in ctx_tiles:
                        ctx_tiles[pair] = [
                            psC.tile([65, QBS], f32, name=f"ctx{h2}", tag=f"ctx{h2}")
                            for h2 in range(2)
                        ]
                    eT2 = eT_store.pop(G)
                    for j in range(KGRP):
                        kc = grp * KGRP + j
                        vsl = slice(kc * 130 + g * 65, kc * 130 + (g + 1) * 65)
                        for h2 in range(2):
                            nc.tensor.matmul(
                                ctx_tiles[pair][h2][:],
                                v_sb[:, vsl],
                                eT2[h2][:, j * QBS : (j + 1) * QBS],
                                start=(kc == 0),
                                stop=(kc == SC - 1),
                            )
                    if grp == NGRP - 1:
                        normalize(pair)

                def normalize(pair):
                    qb, pr = divmod(pair, 4)
                    qsl = slice(qb * QBS, (qb + 1) * QBS)
                    ctxp = ctx_tiles.pop(pair)
                    # copy to SBUF staging right away to release the PSUM
                    # banks for the next pair; normalize off the staging copy
                    stg = [sg.tile([65, QBS], f32, name=f"stg{h2}", tag=f"stg{h2}") for h2 in range(2)]
                    for h2 in range(2):
                        nc.vector.tensor_copy(stg[h2][:], ctxp[h2][:])
                    for h2 in range(2):
                        recip = rp.tile([1, QBS], f32r, tag="recip")
                        with nc.allow_low_precision(reason="f32r is 4-byte"):
                            nc.vector.reciprocal(recip[:], stg[h2][64:65, :])
                        bc = rp.tile([64, QBS], f32r, tag="bc")
                        nc.gpsimd.partition_broadcast(bc[:], recip[:])
                        hs = slice(h2 * 64, (h2 + 1) * 64)
                        nc.vector.tensor_tensor(
                            out=ctxT2[pr][hs, qsl],
                            in0=stg[h2][0:64, :],
                            in1=bc[:],
                            op=ALU.mult,
                        )

                def outproj_quarter(qb, qi):
                    qt = qb * 4 + qi // 4
                    qtr = qi % 4
                    ops = psO.tile([128, 512], f32, tag="out")
                    for p in range(4):
                        nc.tensor.matmul(
                            ops[:],
                            ctxT2[p][:, qt * 128 : (qt + 1) * 128],
                            Wo_sb[p][:, qtr * 512 : (qtr + 1) * 512],
                            start=(p == 0),
                            stop=(p == 3),
                        )
                    osb = so.tile([128, 512], f32, tag="osb")
                    nc.vector.tensor_copy(osb[:], ops[:])
                    nc.sync.dma_start(
                        y[qt * 128 : (qt + 1) * 128, qtr * 512 : (qtr + 1) * 512],
                        osb[:],
                    )

                NG = NPAIR * NGRP  # 128 global groups
                for G in range(NG):
                    pair, grp = divmod(G, NGRP)
                    if G - LAG >= 0:
                        ctx(G - LAG)
                    # q-projection for the next pair, 2 dc chunks per group
                    if pair + 1 < NPAIR:
                        qproj_mms(pair + 1, [2 * grp, 2 * grp + 1])
                        if grp == NGRP - 1:
                            qproj_finish(pair + 1)
                    # out-projection of the previous query block, 1 quarter
                    # per group, spread over the 3rd+4th pairs of this block
                    # (the previous block's last normalize chain is then done)
                    if pair % 4 in (2, 3) and pair >= 6:
                        qi = (pair % 4 - 2) * NGRP + grp
                        outproj_quarter(pair // 4 - 1, qi)
                    scores(G)
                # drain: remaining ctx groups, then last block's out-projection
                for G in range(NG - LAG, NG):
                    ctx(G)
                for qi in range(16):
                    outproj_quarter(3, qi)

    nc.compile()
    return nc


def _get_nc():
    if "nc" not in _CACHE:
        _CACHE["nc"] = _build_bass()
    return _CACHE["nc"]


def make_in_maps(x, Wq, bq, Wk, bk, Wv, bv, Wo):
    f16 = np.float16
    xTb = [np.ascontiguousarray(x[b].T).astype(f16) for b in range(2)]
    Wq16 = Wq.astype(f16)
    Wk16 = Wk.astype(f16)
    Wv16 = Wv.astype(f16)
    Wo16 = Wo.astype(f16)
    in_maps = []
    for c in range(N_CORES):
        b, sh = divmod(c, 4)
        in_maps.append(
            {
                "xT": xTb[b],
                "Wq": np.ascontiguousarray(Wq16[:, sh * CPS : (sh + 1) * CPS]),
                "Wk": np.ascontiguousarray(Wk16[:, sh * KPS : (sh + 1) * KPS]),
                "Wv": np.ascontiguousarray(Wv16[:, sh * KPS : (sh + 1) * KPS]),
                "Wo": np.ascontiguousarray(Wo16[sh * CPS : (sh + 1) * CPS, :]),
                "bq": np.ascontiguousarray(bq[sh * CPS : (sh + 1) * CPS]),
                "bk": np.ascontiguousarray(bk[sh * KPS : (sh + 1) * KPS]),
                "bv": np.ascontiguousarray(bv[sh * KPS : (sh + 1) * KPS]),
            }
        )
    return in_maps


def kernel(x, Wq, bq, Wk, bk, Wv, bv, Wo, bo):
    from concourse.bass_utils import run_bass_kernel_spmd

    x = np.asarray(x, dtype=np.float32)
    Wq = np.asarray(Wq, dtype=np.float32)
    Wk = np.asarray(Wk, dtype=np.float32)
    Wv = np.asarray(Wv, dtype=np.float32)
    Wo = np.asarray(Wo, dtype=np.float32)
    bq = np.asarray(bq, dtype=np.float32)
    bk = np.asarray(bk, dtype=np.float32)
    bv = np.asarray(bv, dtype=np.float32)
    bo = np.asarray(bo, dtype=np.float32)

    in_maps = make_in_maps(x, Wq, bq, Wk, bk, Wv, bv, Wo)
    nc = _get_nc()
    res = run_bass_kernel_spmd(nc, in_maps, core_ids=list(range(N_CORES)))
    out = np.zeros((2, S, D), dtype=np.float32)
    for c in range(N_CORES):
        b = c // 4
        out[b] += res.results[c]["y"]
    out += bo
    return out
